# revision 23
# baseline (speedup 1.0000x reference)
"""Trainium2 Bass kernel for nn_BasisOrbitalBackflow.

Math (reference collapses the N x N pair pooling):
    chi[b,i,mu]   = hermite_prod(ri[b,i], mu) * exp(-0.5 sigma_mu^2 |ri[b,i]|^2)
    S[b,mu]       = sum_i chi[b,i,mu]
    A[b,i,p]      = S[b,p] - chi[b,i,p]
    out[b,i,o]    = sum_{p,q} A[b,i,p] chi[b,i,q] C[p,q,o] / (N-1)

Device strategy (pure data parallel over batch, 8 cores, 256 batches each):
    * C (permuted/scaled) compressed to a rank-128 CP decomposition via ALS
      (generic rank of a 20x20x14 tensor is ~108, so rank 128 fits to ~1e-6):
      C[p,q,o] ~= sum_m U[p,m] V[q,m] Z[m,o]
      -> out^T = Z^T @ ((U^T A^T) * (V^T B^T)); one 128-wide rho chunk.
    * fp16 matmul path; rel err vs the f64 reference ~1e-3 (gate is 2e-2).
    * basis chi built in fp16 on DVE [112 partitions, 32 tiles x 32 mu-slots]
      (Hermite polys rescaled by powers of two; scale folded into C)
    * PE transposes (fp16: 1 cyc/row) move basis into one-bank fp16 PSUM
      tiles [(jb:4)(mu:32)=128p, (cs:4)(112)], one tile per half so the h1
      transposes don't serialize behind h0's consumers; ACT drains B^T to
      fp16 SBUF
    * S = segment-reduce over i (f32 accum), A^T = S_bcast - B^T -> fp16
    * all matmuls use full 128x128 zero-banded weights (no tile_position —
      small weight tiles run at half the column rate on this hardware):
      per group g, U_g/V_g have rows outside [32g, 32g+20) zeroed; Z_g has
      cols outside [32g, 32g+14) zeroed and the four groups accumulate into
      one PSUM tile.
    * ACT drains the V-side PSUM to fp16 SBUF, DVE forms t = GA * GB (fp16)
    * one packed [128, 896] drain (V+ACT halves), output DMA per group pair
"""

import itertools
import numpy as np

N_MAX = 3
SDIM = 3
N_PART = 14
BATCH = 2048
NB = 20
N_CORES = 8
BC = BATCH // N_CORES          # 256 batches per core
R = BC * N_PART                # 3584 rows per core
P = 112                        # rows per tile (8 batches)
T = R // P                     # 32 tiles
G = 4                          # transposed-layout groups (jb)
MS = 32                        # mu slot stride (20 real + 12 pad)
RHO = 128                      # CP rank (one PE chunk)
NWU = 12                       # PE clock warm-up matmuls
NWU2 = 4                       # gap fillers: PE pstate decays during idles
ALS_ITERS = 1200

# ---------------------------------------------------------------------------
# host-side constant construction
# ---------------------------------------------------------------------------

# reference mu ordering (sorted by |n|, stable)
_NS_REF = [tuple(n) for n in sorted(
    (n for n in itertools.product(range(N_MAX + 1), repeat=SDIM) if sum(n) <= N_MAX),
    key=sum)]

# our mu ordering, chosen so the product assembly uses few strided DVE ops.
_PAIRS = [(0, 0), (0, 1), (0, 2), (0, 3), (1, 0), (1, 1), (1, 2), (2, 0), (2, 1), (3, 0)]
_NS_OURS = ([(0,) + pr for pr in _PAIRS]
            + [(1,) + _PAIRS[k] for k in (0, 1, 2, 4, 5, 7)]
            + [(2,) + _PAIRS[k] for k in (0, 1, 4)]
            + [(3, 0, 0)])
assert sorted(_NS_OURS) == sorted(_NS_REF) and len(_NS_OURS) == NB
_PERM = np.array([_NS_REF.index(n) for n in _NS_OURS], dtype=np.int64)  # ours -> ref
_ABS_N = np.array([sum(n) for n in _NS_OURS], dtype=np.float64)


def _cp_als(C, rank, iters, seed=0, reg=1e-12):
    """Rank-`rank` CP decomposition of the (20,20,14) tensor C by ALS."""
    rng = np.random.default_rng(seed)
    U = rng.standard_normal((NB, rank))
    V = rng.standard_normal((NB, rank))
    Z = rng.standard_normal((N_PART, rank))
    C1 = C.reshape(NB, NB * N_PART)
    C2 = C.transpose(1, 0, 2).reshape(NB, NB * N_PART)
    C3 = C.transpose(2, 0, 1).reshape(N_PART, NB * NB)
    eye = reg * np.eye(rank)
    for _ in range(iters):
        KR = (V[:, None, :] * Z[None, :, :]).reshape(NB * N_PART, rank)
        U = np.linalg.solve((V.T @ V) * (Z.T @ Z) + eye, KR.T @ C1.T).T
        KR = (U[:, None, :] * Z[None, :, :]).reshape(NB * N_PART, rank)
        V = np.linalg.solve((U.T @ U) * (Z.T @ Z) + eye, KR.T @ C2.T).T
        KR = (U[:, None, :] * V[None, :, :]).reshape(NB * NB, rank)
        Z = np.linalg.solve((U.T @ U) * (V.T @ V) + eye, KR.T @ C3.T).T
        nu = np.linalg.norm(U, axis=0)
        nv = np.linalg.norm(V, axis=0)
        nz = np.linalg.norm(Z, axis=0)
        g = np.cbrt(nu * nv * nz)
        U *= g / nu
        V *= g / nv
        Z *= g / nz
    fit = np.linalg.norm(np.einsum('pr,qr,or->pqo', U, V, Z) - C) / np.linalg.norm(C)
    return U, V, Z, fit


def _decompose(coeff, sigma):
    """Build all device constants from the (400,14) coeff and (20,) sigma."""
    C = np.asarray(coeff, dtype=np.float64).reshape(NB, NB, N_PART)
    # permute to our mu order; ALS runs on the unscaled tensor (converges to
    # ~1e-6 there), then the 2^{|n|} Hermite rescale folds exactly into the
    # U/V factor rows afterwards.
    C = C[np.ix_(_PERM, _PERM)] / (N_PART - 1)

    U, V, Z, fit = _cp_als(C, RHO, ALS_ITERS)
    if fit > 5e-4:  # rare: try more iterations / another seed
        U, V, Z, fit = _cp_als(C, RHO, 4 * ALS_ITERS, seed=1)
    scale = 2.0 ** _ABS_N
    U = U * scale[:, None]
    V = V * scale[:, None]

    # per-group zero-banded full 128x128 weights, packed [128, (g:4)(128)]
    # on-device (per-partition contiguous on the DRAM side)
    Upad = np.zeros((128, G, RHO))
    Vpad = np.zeros((128, G, RHO))
    Zpad = np.zeros((128, G, 128))
    for g in range(G):
        Upad[MS * g:MS * g + NB, g] = U
        Vpad[MS * g:MS * g + NB, g] = V
        Zpad[:, g, MS * g:MS * g + N_PART] = Z.T

    sig = np.asarray(sigma, dtype=np.float64)
    s2 = -0.5 * (sig[_PERM] ** 2)                       # per-mu, our order
    s2c = np.broadcast_to(s2, (P, NB)).copy()           # replicated to partitions

    return (Upad.astype(np.float16).reshape(128, G * RHO),
            Vpad.astype(np.float16).reshape(128, G * RHO),
            Zpad.astype(np.float16).reshape(128, G * 128),
            s2c.astype(np.float32))


# ---------------------------------------------------------------------------
# device program
# ---------------------------------------------------------------------------

_PROGRAM = None


def _build_program():
    import concourse.bacc as bacc
    import concourse.tile as tile
    import concourse.mybir as mybir
    from concourse._compat import axon_active

    dt = mybir.dt
    f32 = dt.float32
    f16 = dt.float16
    Alu = mybir.AluOpType
    ActF = mybir.ActivationFunctionType

    nc = bacc.Bacc(
        "TRN2",
        target_bir_lowering=False,
        debug=not axon_active(),
        num_devices=N_CORES,
    )

    x_d = nc.dram_tensor("x", [P, T * SDIM], f32, kind="ExternalInput")
    s2c_d = nc.dram_tensor("s2c", [P, NB], f32, kind="ExternalInput")
    id_d = nc.dram_tensor("ident", [P, P], f16, kind="ExternalInput")
    u_d = nc.dram_tensor("u", [128, G * RHO], f16, kind="ExternalInput")
    v_d = nc.dram_tensor("v", [128, G * RHO], f16, kind="ExternalInput")
    z_d = nc.dram_tensor("z", [128, G * 128], f16, kind="ExternalInput")
    out_d = nc.dram_tensor("out_t", [G, N_PART, 2 * 448], f32, kind="ExternalOutput")

    with tile.TileContext(nc) as tc:
        with (
            tc.tile_pool(name="sb", bufs=1) as sb,
            tc.tile_pool(name="ps", bufs=3, space="PSUM") as ps,
            tc.tile_pool(name="po", bufs=1, space="PSUM") as po,
        ):
            htab = sb.tile([P, 4 * T * SDIM], f32, tag="htab")   # (n:4)(t:32)(d:3)
            x2 = sb.tile([P, T * SDIM], f32, tag="x2")
            rho_t = sb.tile([P, T], f32, tag="rho")
            s2c = sb.tile([P, NB], f32, tag="s2c")
            ident = sb.tile([P, P], f16, tag="ident")
            u16 = sb.tile([128, G * RHO], f16, tag="u16")
            v16 = sb.tile([128, G * RHO], f16, tag="v16")
            z16 = sb.tile([128, G * 128], f16, tag="z16")
            hprod = sb.tile([P, T * MS], f32, tag="hprod")       # (t)(m:32)
            earg = sb.tile([P, T * NB], f32, tag="earg")         # (t)(20)
            env = sb.tile([P, T * NB], f32, tag="env")
            basis = sb.tile([P, T * MS], f16, tag="basis")       # (t)(m:32)
            stv = sb.tile([128, 2 * G * 8], f32, tag="stv")      # (h:2)(cs:4)(b:8)
            at16 = sb.tile([128, 2 * 448], f16, tag="at16")      # (h:2)(cs:4)(112)
            bt16 = sb.tile([128, 2 * 448], f16, tag="bt16")
            gb16 = sb.tile([128, 3 * 896], f16, tag="gb16")      # triple buffer
            t16 = sb.tile([128, G * 896], f16, tag="t16")        # (g:4)(h:2)(448)
            osb = sb.tile([128, 2 * 448], f32, tag="osb")        # (h:2)(448)
            wu_w = sb.tile([128, 128], f16, tag="wu_w")
            wu_r = sb.tile([128, 512], f16, tag="wu_r")

            # ---- input DMAs ---------------------------------------------
            # x via the gpsimd software DGE: the GP engine is free right
            # after the start barrier, HWDGE queues only get going ~2us
            # later (scalar is also blocked by the hoisted ACT table load).
            h4 = htab[:].rearrange("p (n t d) -> p n t d", n=4, t=T, d=SDIM)
            xv_src = x_d[:].rearrange("p (t d) -> p t d", t=T, d=SDIM)
            nc.sync.dma_start(h4[0:56, 1], xv_src[0:56])
            nc.gpsimd.dma_start(h4[56:P, 1], xv_src[56:P])
            nc.sync.dma_start(s2c[:], s2c_d[:])
            nc.sync.dma_start(v16[:], v_d[:])
            nc.sync.dma_start(u16[:], u_d[:])
            nc.sync.dma_start(z16[:], z_d[:])
            nc.sync.dma_start(ident[:], id_d[:])

            # ---- memsets (warm-up operands on DVE so the PE can start
            # while gpsimd is still issuing the x DMA) --------------------
            nc.vector.memset(wu_w[:], 1.0)
            nc.vector.memset(wu_r[:], 1.0)
            nc.gpsimd.memset(h4[:, 0], 1.0)
            hp = hprod[:].rearrange("p (t m) -> p t m", t=T, m=MS)
            bb4 = basis[:].rearrange("p (t m) -> p t m", t=T, m=MS)
            nc.gpsimd.memset(hp[:, :, 0], 1.0)
            nc.gpsimd.memset(bb4[:, :, NB:MS], 0.0)   # pad mu slots -> 0

            # ---- PE warm-up (fp16, full-tile like the real matmuls) -----
            wu_p = ps.tile([128, 1024], f32, tag="ps")
            for wi in range(NWU):
                nc.tensor.matmul(wu_p[:, 0:512], wu_w[:], wu_r[:],
                                 start=True, stop=True)

            # ---- hermite table -----------------------------------------
            x_ap = h4[:, 1]
            x2v = x2[:].rearrange("p (t d) -> p t d", t=T, d=SDIM)
            nc.vector.tensor_tensor(x2v, x_ap, x_ap, op=Alu.mult)
            nc.vector.tensor_reduce(rho_t[:], x2v, axis=mybir.AxisListType.X,
                                    op=Alu.add)
            # envelope argument early: the rho -> earg -> exp -> basis chain
            # is the critical path (exp runs on ACT, in parallel with DVE)
            ea = earg[:].rearrange("p (t m) -> p t m", t=T, m=NB)
            ev = env[:].rearrange("p (t m) -> p t m", t=T, m=NB)
            TH = T // 2
            # h2' = x^2 - 0.5   (H2 = 4x^2-2 = 4*h2')
            nc.vector.tensor_scalar_sub(h4[:, 2], x2v, 0.5)
            # h3' = (x^2 - 1.5)*x   (H3 = 8x^3-12x = 8*h3')
            nc.vector.scalar_tensor_tensor(h4[:, 3], x2v, 1.5, x_ap,
                                           op0=Alu.subtract, op1=Alu.mult)
            # mu1..3 copy queued on ACT before the exps: it gates DVE's
            # x0-product chain, the exps are only needed later by bb
            nc.scalar.copy(hp[:, :, 1:4], h4[:, 1:4, :, 2].transpose([0, 2, 1]))
            for h in range(2):
                ts = slice(TH * h, TH * (h + 1))
                eng = nc.vector if h == 0 else nc.gpsimd
                eng.tensor_tensor(
                    ea[:, ts],
                    rho_t[:, ts].unsqueeze(-1).broadcast_to((P, TH, NB)),
                    s2c[:].unsqueeze(1).broadcast_to((P, TH, NB)),
                    op=Alu.mult)
                nc.scalar.activation(ev[:, ts], ea[:, ts], ActF.Exp)

            # ---- pair products into hprod[:, :, 0:10] -------------------
            # mu4..6: h1(x1) * {1, h1(x2), h2'(x2)}
            x1h1 = h4[:, 1, :, 1].unsqueeze(-1).broadcast_to((P, T, 3))
            nc.vector.tensor_tensor(hp[:, :, 4:7], x1h1,
                                    h4[:, 0:3, :, 2].transpose([0, 2, 1]),
                                    op=Alu.mult)
            # mu7..8: h2'(x1) * {1, h1(x2)}
            x1h2 = h4[:, 2, :, 1].unsqueeze(-1).broadcast_to((P, T, 2))
            nc.vector.tensor_tensor(hp[:, :, 7:9], x1h2,
                                    h4[:, 0:2, :, 2].transpose([0, 2, 1]),
                                    op=Alu.mult)
            # mu9: h3'(x1)
            nc.gpsimd.tensor_copy(hp[:, :, 9], h4[:, 3, :, 1])

            # ---- x0 products into hprod[:, :, 10:20] --------------------
            x0h1 = h4[:, 1, :, 0].unsqueeze(-1)
            nc.vector.tensor_tensor(hp[:, :, 10:13],
                                    x0h1.broadcast_to((P, T, 3)),
                                    hp[:, :, 0:3], op=Alu.mult)
            nc.vector.tensor_tensor(hp[:, :, 13:15],
                                    x0h1.broadcast_to((P, T, 2)),
                                    hp[:, :, 4:6], op=Alu.mult)
            nc.vector.tensor_tensor(hp[:, :, 15], x0h1.squeeze(-1),
                                    hp[:, :, 7], op=Alu.mult)
            x0h2 = h4[:, 2, :, 0].unsqueeze(-1)
            nc.vector.tensor_tensor(hp[:, :, 16:18],
                                    x0h2.broadcast_to((P, T, 2)),
                                    hp[:, :, 0:2], op=Alu.mult)
            nc.vector.tensor_tensor(hp[:, :, 18], x0h2.squeeze(-1),
                                    hp[:, :, 4], op=Alu.mult)
            nc.gpsimd.tensor_copy(hp[:, :, 19], h4[:, 3, :, 0])

            # ---- basis = hprod * env (fp16, per h-half) -----------------
            for h in range(2):
                ts = slice(TH * h, TH * (h + 1))
                nc.vector.tensor_tensor(bb4[:, ts, 0:NB], hp[:, ts, 0:NB],
                                        ev[:, ts], op=Alu.mult)

            # ---- PE transpose (fp16: 1 cyc/row, one PSUM bank) ----------
            # chunk cc covers basis cols [128cc, 128cc+128) = tiles 4cc..4cc+3
            # -> btp[(jb:4)(mu:32)=128p, 112*cc ..+112]  (1792B: single bank)
            sv = stv[:].rearrange("p (h c b) -> p h c b", h=2, c=G, b=8)
            av = at16[:].rearrange("p (h c q) -> p h c q", h=2, c=G, q=P)
            bv = bt16[:].rearrange("p (h c q) -> p h c q", h=2, c=G, q=P)
            for h in range(2):
                # separate one-bank PSUM tile per half: no write-after-read
                # coupling between h1 transposes and h0's S/A consumers
                btp = ps.tile([128, 448], f16, tag="ps")
                for cs4 in range(4):
                    cc = 4 * h + cs4
                    nc.tensor.transpose(
                        btp[:, 112 * cs4:112 * cs4 + P],
                        basis[:, 128 * cc:128 * (cc + 1)],
                        ident[:])
                bsrc = btp[:].rearrange("p (c q) -> p c q", c=G, q=P)
                nc.scalar.copy(bv[:, h], bsrc)
                # the reduce reads the PSUM tile directly (starts right after
                # the transposes, not after the ACT copy); the subtract reads
                # the SBUF copy so btp's slot still frees before the
                # B-projections need it
                nc.vector.tensor_reduce(
                    sv[:, h],
                    bsrc.rearrange("p c (b i) -> p c b i", b=8, i=N_PART),
                    axis=mybir.AxisListType.X, op=Alu.add)
                bt_bi = bv[:, h].rearrange("p c (b i) -> p c b i", b=8, i=N_PART)
                nc.vector.tensor_tensor(
                    av[:, h].rearrange("p c (b i) -> p c b i", b=8, i=N_PART),
                    sv[:, h].unsqueeze(-1).broadcast_to((128, G, 8, N_PART)),
                    bt_bi, op=Alu.subtract)
            # keep the PE clock hot through the S/A window
            for _ in range(NWU2):
                nc.tensor.matmul(wu_p[:, 0:512], wu_w[:], wu_r[:],
                                 start=True, stop=True)

            # ---- rank projections, product, output projection -----------
            # all weights are full 128x128 zero-banded tiles; the four output
            # groups accumulate into one PSUM tile (bank per h).
            o_ps = po.tile([128, 1024], f32, tag="po")

            def b_proj(g):
                b_ps = ps.tile([128, 1024], f32, tag="ps")
                for h in range(2):
                    cs = slice(448 * h, 448 * (h + 1))
                    nc.tensor.matmul(b_ps[:, 512 * h:512 * h + 448],
                                     v16[:, RHO * g:RHO * (g + 1)], bt16[:, cs],
                                     start=True, stop=True)
                gbv = gb16[:, 896 * (g % 3):896 * (g % 3) + 896].rearrange(
                    "p (h q) -> p h q", h=2, q=448)
                bp2 = b_ps[:].rearrange("p (h q) -> p h q", h=2, q=512)[:, :, 0:448]
                nc.scalar.copy(gbv, bp2)
                return gbv

            # B-projections run two groups ahead so the ACT drains pipeline
            # in front of the DVE products instead of starving them
            gbs = [b_proj(0), b_proj(1)]
            for g in range(G):
                a_ps = ps.tile([128, 1024], f32, tag="ps")
                for h in range(2):
                    cs = slice(448 * h, 448 * (h + 1))
                    nc.tensor.matmul(a_ps[:, 512 * h:512 * h + 448],
                                     u16[:, RHO * g:RHO * (g + 1)], at16[:, cs],
                                     start=True, stop=True)
                if g + 2 < G:
                    gbs.append(b_proj(g + 2))
                ap2 = a_ps[:].rearrange("p (h q) -> p h q", h=2, q=512)[:, :, 0:448]
                tg = t16[:, 896 * g:896 * (g + 1)].rearrange(
                    "p (h q) -> p h q", h=2, q=448)
                nc.vector.tensor_tensor(tg, ap2, gbs[g], op=Alu.mult)
                # out^T per group at PSUM partitions 32g..32g+13: lets each
                # group drain and DMA out while later products still run
                zg = z16[:, 128 * g + MS * g:][:, 0:N_PART]
                for h in range(2):
                    nc.tensor.matmul(o_ps[MS * g:MS * g + N_PART,
                                          512 * h:512 * h + 448],
                                     zg, t16[:, 896 * g + 448 * h:][:, 0:448],
                                     start=True, stop=True,
                                     tile_position=(0, MS * g))
                ovg = o_ps[MS * g:MS * g + N_PART].rearrange(
                    "p (h q) -> p h q", h=2, q=512)[:, :, 0:448]
                osg = osb[MS * g:MS * g + N_PART].rearrange(
                    "p (h q) -> p h q", h=2, q=448)
                nc.scalar.copy(osg, ovg)
                engs = [nc.gpsimd, nc.sync, nc.scalar, nc.gpsimd]
                engs[g].dma_start(out_d[g], osb[MS * g:MS * g + N_PART, :])

    nc.compile()
    return nc


def _get_program():
    global _PROGRAM
    if _PROGRAM is None:
        _PROGRAM = _build_program()
    return _PROGRAM


# ---------------------------------------------------------------------------
# entry point
# ---------------------------------------------------------------------------

LAST_RESULTS = None


def kernel(ri, rij_dist=None, sigma=None, coeff=None, **_unused):
    import os
    from concourse.bass_utils import run_bass_kernel_spmd

    global LAST_RESULTS
    ri = np.ascontiguousarray(np.asarray(ri, dtype=np.float32))
    U, V, Zd, s2c = _decompose(coeff, sigma)
    ident = np.eye(P, dtype=np.float16)

    nc = _get_program()
    in_maps = []
    for i in range(N_CORES):
        chunk = ri[i * BC:(i + 1) * BC].reshape(T, P, SDIM)
        x = np.ascontiguousarray(chunk.transpose(1, 0, 2).reshape(P, T * SDIM))
        in_maps.append({
            "x": x, "s2c": s2c, "u": U, "v": V, "z": Zd, "ident": ident,
        })

    trace = bool(int(os.environ.get("BOB_TRACE", "0")))
    res = run_bass_kernel_spmd(nc, in_maps, core_ids=list(range(N_CORES)),
                               trace=trace)
    LAST_RESULTS = res

    outs = []
    for i in range(N_CORES):
        ot = res.results[i]["out_t"]                      # (4, 14, 896)
        # col c = (h:2)(cs:4)(p:112); tile t = 4*(4h+cs) + g = 16h + 4cs + g
        arr = ot.reshape(G, N_PART, 2, 4, P).transpose(2, 3, 0, 4, 1)
        outs.append(arr.reshape(R, N_PART).reshape(BC, N_PART, N_PART))
    return np.ascontiguousarray(np.concatenate(outs, axis=0), dtype=np.float32)


# revision 25
# speedup vs baseline: 1.0217x; 1.0217x over previous
"""Trainium2 Bass kernel for nn_BasisOrbitalBackflow.

Math (reference collapses the N x N pair pooling):
    chi[b,i,mu]   = hermite_prod(ri[b,i], mu) * exp(-0.5 sigma_mu^2 |ri[b,i]|^2)
    S[b,mu]       = sum_i chi[b,i,mu]
    A[b,i,p]      = S[b,p] - chi[b,i,p]
    out[b,i,o]    = sum_{p,q} A[b,i,p] chi[b,i,q] C[p,q,o] / (N-1)

Device strategy (pure data parallel over batch, 8 cores, 256 batches each):
    * C (permuted/scaled) compressed to a rank-128 CP decomposition via ALS
      (generic rank of a 20x20x14 tensor is ~108, so rank 128 fits to ~1e-6):
      C[p,q,o] ~= sum_m U[p,m] V[q,m] Z[m,o]
      -> out^T = Z^T @ ((U^T A^T) * (V^T B^T)); one 128-wide rho chunk.
    * fp16 matmul path; rel err vs the f64 reference ~1e-3 (gate is 2e-2).
    * basis chi built in fp16 on DVE [112 partitions, 32 tiles x 32 mu-slots]
      (Hermite polys rescaled by powers of two; scale folded into C)
    * PE transposes (fp16: 1 cyc/row) move basis into one-bank fp16 PSUM
      tiles [(jb:4)(mu:32)=128p, (cs:4)(112)], one tile per half so the h1
      transposes don't serialize behind h0's consumers; ACT drains B^T to
      fp16 SBUF
    * S = segment-reduce over i (f32 accum), A^T = S_bcast - B^T -> fp16
    * all matmuls use full 128x128 zero-banded weights (no tile_position —
      small weight tiles run at half the column rate on this hardware):
      per group g, U_g/V_g have rows outside [32g, 32g+20) zeroed; Z_g has
      cols outside [32g, 32g+14) zeroed and the four groups accumulate into
      one PSUM tile.
    * ACT drains the V-side PSUM to fp16 SBUF, DVE forms t = GA * GB (fp16)
    * one packed [128, 896] drain (V+ACT halves), output DMA per group pair
"""

import itertools
import numpy as np

N_MAX = 3
SDIM = 3
N_PART = 14
BATCH = 2048
NB = 20
N_CORES = 8
BC = BATCH // N_CORES          # 256 batches per core
R = BC * N_PART                # 3584 rows per core
P = 112                        # rows per tile (8 batches)
T = R // P                     # 32 tiles
G = 4                          # transposed-layout groups (jb)
MS = 32                        # mu slot stride (20 real + 12 pad)
RHO = 128                      # CP rank (one PE chunk)
NWU = 12                       # PE clock warm-up matmuls
NWU2 = 4                       # gap fillers: PE pstate decays during idles
ALS_ITERS = 1200

# ---------------------------------------------------------------------------
# host-side constant construction
# ---------------------------------------------------------------------------

# reference mu ordering (sorted by |n|, stable)
_NS_REF = [tuple(n) for n in sorted(
    (n for n in itertools.product(range(N_MAX + 1), repeat=SDIM) if sum(n) <= N_MAX),
    key=sum)]

# our mu ordering, chosen so the product assembly uses few strided DVE ops.
_PAIRS = [(0, 0), (0, 1), (0, 2), (0, 3), (1, 0), (1, 1), (1, 2), (2, 0), (2, 1), (3, 0)]
_NS_OURS = ([(0,) + pr for pr in _PAIRS]
            + [(1,) + _PAIRS[k] for k in (0, 1, 2, 4, 5, 7)]
            + [(2,) + _PAIRS[k] for k in (0, 1, 4)]
            + [(3, 0, 0)])
assert sorted(_NS_OURS) == sorted(_NS_REF) and len(_NS_OURS) == NB
_PERM = np.array([_NS_REF.index(n) for n in _NS_OURS], dtype=np.int64)  # ours -> ref
_ABS_N = np.array([sum(n) for n in _NS_OURS], dtype=np.float64)


def _cp_als(C, rank, iters, seed=0, reg=1e-12):
    """Rank-`rank` CP decomposition of the (20,20,14) tensor C by ALS."""
    rng = np.random.default_rng(seed)
    U = rng.standard_normal((NB, rank))
    V = rng.standard_normal((NB, rank))
    Z = rng.standard_normal((N_PART, rank))
    C1 = C.reshape(NB, NB * N_PART)
    C2 = C.transpose(1, 0, 2).reshape(NB, NB * N_PART)
    C3 = C.transpose(2, 0, 1).reshape(N_PART, NB * NB)
    eye = reg * np.eye(rank)
    for _ in range(iters):
        KR = (V[:, None, :] * Z[None, :, :]).reshape(NB * N_PART, rank)
        U = np.linalg.solve((V.T @ V) * (Z.T @ Z) + eye, KR.T @ C1.T).T
        KR = (U[:, None, :] * Z[None, :, :]).reshape(NB * N_PART, rank)
        V = np.linalg.solve((U.T @ U) * (Z.T @ Z) + eye, KR.T @ C2.T).T
        KR = (U[:, None, :] * V[None, :, :]).reshape(NB * NB, rank)
        Z = np.linalg.solve((U.T @ U) * (V.T @ V) + eye, KR.T @ C3.T).T
        nu = np.linalg.norm(U, axis=0)
        nv = np.linalg.norm(V, axis=0)
        nz = np.linalg.norm(Z, axis=0)
        g = np.cbrt(nu * nv * nz)
        U *= g / nu
        V *= g / nv
        Z *= g / nz
    fit = np.linalg.norm(np.einsum('pr,qr,or->pqo', U, V, Z) - C) / np.linalg.norm(C)
    return U, V, Z, fit


def _decompose(coeff, sigma):
    """Build all device constants from the (400,14) coeff and (20,) sigma."""
    C = np.asarray(coeff, dtype=np.float64).reshape(NB, NB, N_PART)
    # permute to our mu order; ALS runs on the unscaled tensor (converges to
    # ~1e-6 there), then the 2^{|n|} Hermite rescale folds exactly into the
    # U/V factor rows afterwards.
    C = C[np.ix_(_PERM, _PERM)] / (N_PART - 1)

    U, V, Z, fit = _cp_als(C, RHO, ALS_ITERS)
    if fit > 5e-4:  # rare: try more iterations / another seed
        U, V, Z, fit = _cp_als(C, RHO, 4 * ALS_ITERS, seed=1)
    scale = 2.0 ** _ABS_N
    U = U * scale[:, None]
    V = V * scale[:, None]

    # per-group zero-banded full 128x128 weights, packed [128, (g:4)(128)]
    # on-device (per-partition contiguous on the DRAM side)
    Upad = np.zeros((128, G, RHO))
    Vpad = np.zeros((128, G, RHO))
    Zpad = np.zeros((128, G, 128))
    for g in range(G):
        Upad[MS * g:MS * g + NB, g] = U
        Vpad[MS * g:MS * g + NB, g] = V
        Zpad[:, g, MS * g:MS * g + N_PART] = Z.T

    sig = np.asarray(sigma, dtype=np.float64)
    s2 = -0.5 * (sig[_PERM] ** 2)                       # per-mu, our order
    s2c = np.broadcast_to(s2, (P, NB)).copy()           # replicated to partitions

    return (Upad.astype(np.float16).reshape(128, G * RHO),
            Vpad.astype(np.float16).reshape(128, G * RHO),
            Zpad.astype(np.float16).reshape(128, G * 128),
            s2c.astype(np.float32))


# ---------------------------------------------------------------------------
# device program
# ---------------------------------------------------------------------------

_PROGRAM = None


def _build_program():
    import concourse.bacc as bacc
    import concourse.tile as tile
    import concourse.mybir as mybir
    from concourse._compat import axon_active

    dt = mybir.dt
    f32 = dt.float32
    f16 = dt.float16
    Alu = mybir.AluOpType
    ActF = mybir.ActivationFunctionType

    nc = bacc.Bacc(
        "TRN2",
        target_bir_lowering=False,
        debug=not axon_active(),
        num_devices=N_CORES,
    )

    x_d = nc.dram_tensor("x", [P, T * SDIM], f32, kind="ExternalInput")
    s2c_d = nc.dram_tensor("s2c", [P, NB], f32, kind="ExternalInput")
    id_d = nc.dram_tensor("ident", [P, P], f16, kind="ExternalInput")
    u_d = nc.dram_tensor("u", [128, G * RHO], f16, kind="ExternalInput")
    v_d = nc.dram_tensor("v", [128, G * RHO], f16, kind="ExternalInput")
    z_d = nc.dram_tensor("z", [128, G * 128], f16, kind="ExternalInput")
    out_d = nc.dram_tensor("out_t", [G, N_PART, 2 * 448], f32, kind="ExternalOutput")

    with tile.TileContext(nc) as tc:
        with (
            tc.tile_pool(name="sb", bufs=1) as sb,
            tc.tile_pool(name="ps", bufs=3, space="PSUM") as ps,
            tc.tile_pool(name="po", bufs=1, space="PSUM") as po,
        ):
            htab = sb.tile([P, 4 * T * SDIM], f32, tag="htab")   # (n:4)(t:32)(d:3)
            x2 = sb.tile([P, T * SDIM], f32, tag="x2")
            rho_t = sb.tile([P, T], f32, tag="rho")
            s2c = sb.tile([P, NB], f32, tag="s2c")
            ident = sb.tile([P, P], f16, tag="ident")
            u16 = sb.tile([128, G * RHO], f16, tag="u16")
            v16 = sb.tile([128, G * RHO], f16, tag="v16")
            z16 = sb.tile([128, G * 128], f16, tag="z16")
            hprod = sb.tile([P, T * MS], f32, tag="hprod")       # (t)(m:32)
            earg = sb.tile([P, T * NB], f32, tag="earg")         # (t)(20)
            env = sb.tile([P, T * NB], f32, tag="env")
            basis = sb.tile([P, T * MS], f16, tag="basis")       # (t)(m:32)
            stv = sb.tile([128, 2 * G * 8], f32, tag="stv")      # (h:2)(cs:4)(b:8)
            at16 = sb.tile([128, 2 * 448], f16, tag="at16")      # (h:2)(cs:4)(112)
            bt16 = sb.tile([128, 2 * 448], f16, tag="bt16")
            gb16 = sb.tile([128, 3 * 896], f16, tag="gb16")      # triple buffer
            t16 = sb.tile([128, G * 896], f16, tag="t16")        # (g:4)(h:2)(448)
            osb = sb.tile([128, 2 * 448], f32, tag="osb")        # (h:2)(448)
            wu_w = sb.tile([128, 128], f16, tag="wu_w")
            wu_r = sb.tile([128, 512], f16, tag="wu_r")

            # ---- input DMAs ---------------------------------------------
            # x via the gpsimd software DGE: the GP engine is free right
            # after the start barrier, HWDGE queues only get going ~2us
            # later (scalar is also blocked by the hoisted ACT table load).
            h4 = htab[:].rearrange("p (n t d) -> p n t d", n=4, t=T, d=SDIM)
            xv_src = x_d[:].rearrange("p (t d) -> p t d", t=T, d=SDIM)
            nc.sync.dma_start(h4[0:56, 1], xv_src[0:56])
            nc.gpsimd.dma_start(h4[56:P, 1], xv_src[56:P])
            nc.sync.dma_start(s2c[:], s2c_d[:])
            nc.sync.dma_start(v16[:], v_d[:])
            nc.sync.dma_start(u16[:], u_d[:])
            nc.sync.dma_start(z16[:], z_d[:])
            nc.sync.dma_start(ident[:], id_d[:])

            # ---- memsets (warm-up operands on DVE so the PE can start
            # while gpsimd is still issuing the x DMA) --------------------
            nc.vector.memset(wu_w[:], 1.0)
            nc.vector.memset(wu_r[:], 1.0)
            nc.gpsimd.memset(h4[:, 0], 1.0)
            hp = hprod[:].rearrange("p (t m) -> p t m", t=T, m=MS)
            bb4 = basis[:].rearrange("p (t m) -> p t m", t=T, m=MS)
            nc.gpsimd.memset(hp[:, :, 0], 1.0)
            nc.gpsimd.memset(bb4[:, :, NB:MS], 0.0)   # pad mu slots -> 0

            # ---- PE warm-up (fp16, full-tile like the real matmuls) -----
            wu_p = ps.tile([128, 1024], f32, tag="ps")
            for wi in range(NWU):
                nc.tensor.matmul(wu_p[:, 0:512], wu_w[:], wu_r[:],
                                 start=True, stop=True)

            # ---- hermite table -----------------------------------------
            x_ap = h4[:, 1]
            x2v = x2[:].rearrange("p (t d) -> p t d", t=T, d=SDIM)
            nc.vector.tensor_tensor(x2v, x_ap, x_ap, op=Alu.mult)
            nc.vector.tensor_reduce(rho_t[:], x2v, axis=mybir.AxisListType.X,
                                    op=Alu.add)
            # envelope argument early: the rho -> earg -> exp -> basis chain
            # is the critical path (exp runs on ACT, in parallel with DVE)
            ea = earg[:].rearrange("p (t m) -> p t m", t=T, m=NB)
            ev = env[:].rearrange("p (t m) -> p t m", t=T, m=NB)
            TH = T // 2
            # h2' = x^2 - 0.5   (H2 = 4x^2-2 = 4*h2')
            nc.vector.tensor_scalar_sub(h4[:, 2], x2v, 0.5)
            # h3' = (x^2 - 1.5)*x   (H3 = 8x^3-12x = 8*h3')
            nc.vector.scalar_tensor_tensor(h4[:, 3], x2v, 1.5, x_ap,
                                           op0=Alu.subtract, op1=Alu.mult)
            # mu1..3 copy queued on ACT before the exps: it gates DVE's
            # x0-product chain, the exps are only needed later by bb
            nc.scalar.copy(hp[:, :, 1:4], h4[:, 1:4, :, 2].transpose([0, 2, 1]))
            for h in range(2):
                ts = slice(TH * h, TH * (h + 1))
                eng = nc.vector if h == 0 else nc.gpsimd
                eng.tensor_tensor(
                    ea[:, ts],
                    rho_t[:, ts].unsqueeze(-1).broadcast_to((P, TH, NB)),
                    s2c[:].unsqueeze(1).broadcast_to((P, TH, NB)),
                    op=Alu.mult)
                nc.scalar.activation(ev[:, ts], ea[:, ts], ActF.Exp)

            # ---- pair products into hprod[:, :, 0:10] -------------------
            # mu4..6: h1(x1) * {1, h1(x2), h2'(x2)}
            x1h1 = h4[:, 1, :, 1].unsqueeze(-1).broadcast_to((P, T, 3))
            nc.vector.tensor_tensor(hp[:, :, 4:7], x1h1,
                                    h4[:, 0:3, :, 2].transpose([0, 2, 1]),
                                    op=Alu.mult)
            # mu7..8: h2'(x1) * {1, h1(x2)}
            x1h2 = h4[:, 2, :, 1].unsqueeze(-1).broadcast_to((P, T, 2))
            nc.vector.tensor_tensor(hp[:, :, 7:9], x1h2,
                                    h4[:, 0:2, :, 2].transpose([0, 2, 1]),
                                    op=Alu.mult)
            # mu9: h3'(x1)
            nc.gpsimd.tensor_copy(hp[:, :, 9], h4[:, 3, :, 1])

            # ---- x0 products into hprod[:, :, 10:20] --------------------
            x0h1 = h4[:, 1, :, 0].unsqueeze(-1)
            nc.vector.tensor_tensor(hp[:, :, 10:13],
                                    x0h1.broadcast_to((P, T, 3)),
                                    hp[:, :, 0:3], op=Alu.mult)
            nc.vector.tensor_tensor(hp[:, :, 13:15],
                                    x0h1.broadcast_to((P, T, 2)),
                                    hp[:, :, 4:6], op=Alu.mult)
            nc.vector.tensor_tensor(hp[:, :, 15], x0h1.squeeze(-1),
                                    hp[:, :, 7], op=Alu.mult)
            x0h2 = h4[:, 2, :, 0].unsqueeze(-1)
            nc.vector.tensor_tensor(hp[:, :, 16:18],
                                    x0h2.broadcast_to((P, T, 2)),
                                    hp[:, :, 0:2], op=Alu.mult)
            nc.vector.tensor_tensor(hp[:, :, 18], x0h2.squeeze(-1),
                                    hp[:, :, 4], op=Alu.mult)
            nc.gpsimd.tensor_copy(hp[:, :, 19], h4[:, 3, :, 0])

            # ---- basis = hprod * env (fp16, per h-half) -----------------
            for h in range(2):
                ts = slice(TH * h, TH * (h + 1))
                nc.vector.tensor_tensor(bb4[:, ts, 0:NB], hp[:, ts, 0:NB],
                                        ev[:, ts], op=Alu.mult)

            # ---- PE transpose (fp16: 1 cyc/row, one PSUM bank) ----------
            # chunk cc covers basis cols [128cc, 128cc+128) = tiles 4cc..4cc+3
            # -> btp[(jb:4)(mu:32)=128p, 112*cc ..+112]  (1792B: single bank)
            sv = stv[:].rearrange("p (h c b) -> p h c b", h=2, c=G, b=8)
            av = at16[:].rearrange("p (h c q) -> p h c q", h=2, c=G, q=P)
            bv = bt16[:].rearrange("p (h c q) -> p h c q", h=2, c=G, q=P)
            for h in range(2):
                # separate one-bank PSUM tile per half: no write-after-read
                # coupling between h1 transposes and h0's S/A consumers
                btp = ps.tile([128, 448], f16, tag="ps")
                for cs4 in range(4):
                    cc = 4 * h + cs4
                    nc.tensor.transpose(
                        btp[:, 112 * cs4:112 * cs4 + P],
                        basis[:, 128 * cc:128 * (cc + 1)],
                        ident[:])
                bsrc = btp[:].rearrange("p (c q) -> p c q", c=G, q=P)
                nc.scalar.copy(bv[:, h], bsrc)
                # the reduce reads the PSUM tile directly, so it starts right
                # after the transposes instead of after the ACT copy; the
                # subtract reads the SBUF copy, so btp's last reader is the
                # reduce and its pool slot still frees before the g1
                # B-projection needs it
                nc.vector.tensor_reduce(
                    sv[:, h],
                    bsrc.rearrange("p c (b i) -> p c b i", b=8, i=N_PART),
                    axis=mybir.AxisListType.X, op=Alu.add)
                nc.vector.tensor_tensor(
                    av[:, h].rearrange("p c (b i) -> p c b i", b=8, i=N_PART),
                    sv[:, h].unsqueeze(-1).broadcast_to((128, G, 8, N_PART)),
                    bv[:, h].rearrange("p c (b i) -> p c b i", b=8, i=N_PART),
                    op=Alu.subtract)
            # keep the PE clock hot through the S/A window
            for _ in range(NWU2):
                nc.tensor.matmul(wu_p[:, 0:512], wu_w[:], wu_r[:],
                                 start=True, stop=True)

            # ---- rank projections, product, output projection -----------
            # all weights are full 128x128 zero-banded tiles; the four output
            # groups accumulate into one PSUM tile (bank per h).
            o_ps = po.tile([128, 1024], f32, tag="po")

            def b_proj(g):
                b_ps = ps.tile([128, 1024], f32, tag="ps")
                for h in range(2):
                    cs = slice(448 * h, 448 * (h + 1))
                    nc.tensor.matmul(b_ps[:, 512 * h:512 * h + 448],
                                     v16[:, RHO * g:RHO * (g + 1)], bt16[:, cs],
                                     start=True, stop=True)
                gbv = gb16[:, 896 * (g % 3):896 * (g % 3) + 896].rearrange(
                    "p (h q) -> p h q", h=2, q=448)
                bp2 = b_ps[:].rearrange("p (h q) -> p h q", h=2, q=512)[:, :, 0:448]
                nc.scalar.copy(gbv, bp2)
                return gbv

            # B-projections run two groups ahead so the ACT drains pipeline
            # in front of the DVE products instead of starving them
            gbs = [b_proj(0), b_proj(1)]
            for g in range(G):
                a_ps = ps.tile([128, 1024], f32, tag="ps")
                for h in range(2):
                    cs = slice(448 * h, 448 * (h + 1))
                    nc.tensor.matmul(a_ps[:, 512 * h:512 * h + 448],
                                     u16[:, RHO * g:RHO * (g + 1)], at16[:, cs],
                                     start=True, stop=True)
                if g + 2 < G:
                    gbs.append(b_proj(g + 2))
                ap2 = a_ps[:].rearrange("p (h q) -> p h q", h=2, q=512)[:, :, 0:448]
                tg = t16[:, 896 * g:896 * (g + 1)].rearrange(
                    "p (h q) -> p h q", h=2, q=448)
                nc.vector.tensor_tensor(tg, ap2, gbs[g], op=Alu.mult)
                # out^T: the four groups accumulate (banded Z columns)
                for h in range(2):
                    nc.tensor.matmul(o_ps[:, 512 * h:512 * h + 448],
                                     z16[:, 128 * g:128 * (g + 1)],
                                     t16[:, 896 * g + 448 * h:][:, 0:448],
                                     start=(g == 0), stop=(g == G - 1))

            # ---- drain + store ------------------------------------------
            ov = o_ps[:].rearrange("p (h q) -> p h q", h=2, q=512)[:, :, 0:448]
            osv = osb[:].rearrange("p (h q) -> p h q", h=2, q=448)
            nc.vector.tensor_copy(osv[:, 0], ov[:, 0])
            nc.scalar.copy(osv[:, 1], ov[:, 1])
            engs = [nc.gpsimd, nc.sync, nc.scalar, nc.scalar]
            for g in range(G):
                engs[g].dma_start(out_d[g], osb[MS * g:MS * g + N_PART, :])

    nc.compile()
    return nc


def _get_program():
    global _PROGRAM
    if _PROGRAM is None:
        _PROGRAM = _build_program()
    return _PROGRAM


# ---------------------------------------------------------------------------
# entry point
# ---------------------------------------------------------------------------

LAST_RESULTS = None


def kernel(ri, rij_dist=None, sigma=None, coeff=None, **_unused):
    import os
    from concourse.bass_utils import run_bass_kernel_spmd

    global LAST_RESULTS
    ri = np.ascontiguousarray(np.asarray(ri, dtype=np.float32))
    U, V, Zd, s2c = _decompose(coeff, sigma)
    ident = np.eye(P, dtype=np.float16)

    nc = _get_program()
    in_maps = []
    for i in range(N_CORES):
        chunk = ri[i * BC:(i + 1) * BC].reshape(T, P, SDIM)
        x = np.ascontiguousarray(chunk.transpose(1, 0, 2).reshape(P, T * SDIM))
        in_maps.append({
            "x": x, "s2c": s2c, "u": U, "v": V, "z": Zd, "ident": ident,
        })

    trace = bool(int(os.environ.get("BOB_TRACE", "0")))
    res = run_bass_kernel_spmd(nc, in_maps, core_ids=list(range(N_CORES)),
                               trace=trace)
    LAST_RESULTS = res

    outs = []
    for i in range(N_CORES):
        ot = res.results[i]["out_t"]                      # (4, 14, 896)
        # col c = (h:2)(cs:4)(p:112); tile t = 4*(4h+cs) + g = 16h + 4cs + g
        arr = ot.reshape(G, N_PART, 2, 4, P).transpose(2, 3, 0, 4, 1)
        outs.append(arr.reshape(R, N_PART).reshape(BC, N_PART, N_PART))
    return np.ascontiguousarray(np.concatenate(outs, axis=0), dtype=np.float32)


# revision 26
# speedup vs baseline: 1.0513x; 1.0291x over previous
"""Trainium2 Bass kernel for nn_BasisOrbitalBackflow.

Math (reference collapses the N x N pair pooling):
    chi[b,i,mu]   = hermite_prod(ri[b,i], mu) * exp(-0.5 sigma_mu^2 |ri[b,i]|^2)
    S[b,mu]       = sum_i chi[b,i,mu]
    A[b,i,p]      = S[b,p] - chi[b,i,p]
    out[b,i,o]    = sum_{p,q} A[b,i,p] chi[b,i,q] C[p,q,o] / (N-1)

Device strategy (pure data parallel over batch, 8 cores, 256 batches each):
    * C (permuted/scaled) compressed to a rank-128 CP decomposition via ALS
      (generic rank of a 20x20x14 tensor is ~108, so rank 128 fits to ~1e-6):
      C[p,q,o] ~= sum_m U[p,m] V[q,m] Z[m,o]
      -> out^T = Z^T @ ((U^T A^T) * (V^T B^T)); one 128-wide rho chunk.
    * fp16 matmul path; rel err vs the f64 reference ~1e-3 (gate is 2e-2).
    * basis chi built in fp16 on DVE [112 partitions, 32 tiles x 32 mu-slots]
      (Hermite polys rescaled by powers of two; scale folded into C)
    * PE transposes (fp16: 1 cyc/row) move basis into one-bank fp16 PSUM
      tiles [(jb:4)(mu:32)=128p, (cs:4)(112)], one tile per half so the h1
      transposes don't serialize behind h0's consumers; ACT drains B^T to
      fp16 SBUF
    * S = segment-reduce over i (f32 accum), A^T = S_bcast - B^T -> fp16
    * all matmuls use full 128x128 zero-banded weights (no tile_position —
      small weight tiles run at half the column rate on this hardware):
      per group g, U_g/V_g have rows outside [32g, 32g+20) zeroed; Z_g has
      cols outside [32g, 32g+14) zeroed and the four groups accumulate into
      one PSUM tile.
    * ACT drains the V-side PSUM to fp16 SBUF, DVE forms t = GA * GB (fp16)
    * one packed [128, 896] drain (V+ACT halves), output DMA per group pair
"""

import itertools
import numpy as np

N_MAX = 3
SDIM = 3
N_PART = 14
BATCH = 2048
NB = 20
N_CORES = 8
BC = BATCH // N_CORES          # 256 batches per core
R = BC * N_PART                # 3584 rows per core
P = 112                        # rows per tile (8 batches)
T = R // P                     # 32 tiles
G = 4                          # transposed-layout groups (jb)
MS = 32                        # mu slot stride (20 real + 12 pad)
RHO = 128                      # CP rank (one PE chunk)
NWU = 12                       # PE clock warm-up matmuls
NWU2 = 4                       # gap fillers: PE pstate decays during idles
ALS_ITERS = 1200

# ---------------------------------------------------------------------------
# host-side constant construction
# ---------------------------------------------------------------------------

# reference mu ordering (sorted by |n|, stable)
_NS_REF = [tuple(n) for n in sorted(
    (n for n in itertools.product(range(N_MAX + 1), repeat=SDIM) if sum(n) <= N_MAX),
    key=sum)]

# our mu ordering, chosen so the product assembly uses few strided DVE ops.
_PAIRS = [(0, 0), (0, 1), (0, 2), (0, 3), (1, 0), (1, 1), (1, 2), (2, 0), (2, 1), (3, 0)]
_NS_OURS = ([(0,) + pr for pr in _PAIRS]
            + [(1,) + _PAIRS[k] for k in (0, 1, 2, 4, 5, 7)]
            + [(2,) + _PAIRS[k] for k in (0, 1, 4)]
            + [(3, 0, 0)])
assert sorted(_NS_OURS) == sorted(_NS_REF) and len(_NS_OURS) == NB
_PERM = np.array([_NS_REF.index(n) for n in _NS_OURS], dtype=np.int64)  # ours -> ref
_ABS_N = np.array([sum(n) for n in _NS_OURS], dtype=np.float64)


def _cp_als(C, rank, iters, seed=0, reg=1e-12):
    """Rank-`rank` CP decomposition of the (20,20,14) tensor C by ALS."""
    rng = np.random.default_rng(seed)
    U = rng.standard_normal((NB, rank))
    V = rng.standard_normal((NB, rank))
    Z = rng.standard_normal((N_PART, rank))
    C1 = C.reshape(NB, NB * N_PART)
    C2 = C.transpose(1, 0, 2).reshape(NB, NB * N_PART)
    C3 = C.transpose(2, 0, 1).reshape(N_PART, NB * NB)
    eye = reg * np.eye(rank)
    for _ in range(iters):
        KR = (V[:, None, :] * Z[None, :, :]).reshape(NB * N_PART, rank)
        U = np.linalg.solve((V.T @ V) * (Z.T @ Z) + eye, KR.T @ C1.T).T
        KR = (U[:, None, :] * Z[None, :, :]).reshape(NB * N_PART, rank)
        V = np.linalg.solve((U.T @ U) * (Z.T @ Z) + eye, KR.T @ C2.T).T
        KR = (U[:, None, :] * V[None, :, :]).reshape(NB * NB, rank)
        Z = np.linalg.solve((U.T @ U) * (V.T @ V) + eye, KR.T @ C3.T).T
        nu = np.linalg.norm(U, axis=0)
        nv = np.linalg.norm(V, axis=0)
        nz = np.linalg.norm(Z, axis=0)
        g = np.cbrt(nu * nv * nz)
        U *= g / nu
        V *= g / nv
        Z *= g / nz
    fit = np.linalg.norm(np.einsum('pr,qr,or->pqo', U, V, Z) - C) / np.linalg.norm(C)
    return U, V, Z, fit


def _decompose(coeff, sigma):
    """Build all device constants from the (400,14) coeff and (20,) sigma."""
    C = np.asarray(coeff, dtype=np.float64).reshape(NB, NB, N_PART)
    # permute to our mu order; ALS runs on the unscaled tensor (converges to
    # ~1e-6 there), then the 2^{|n|} Hermite rescale folds exactly into the
    # U/V factor rows afterwards.
    C = C[np.ix_(_PERM, _PERM)] / (N_PART - 1)

    U, V, Z, fit = _cp_als(C, RHO, ALS_ITERS)
    if fit > 5e-4:  # rare: try more iterations / another seed
        U, V, Z, fit = _cp_als(C, RHO, 4 * ALS_ITERS, seed=1)
    scale = 2.0 ** _ABS_N
    U = U * scale[:, None]
    V = V * scale[:, None]

    # per-group zero-banded full 128x128 weights, packed [128, (g:4)(128)]
    # on-device (per-partition contiguous on the DRAM side)
    Upad = np.zeros((128, G, RHO))
    Vpad = np.zeros((128, G, RHO))
    Zpad = np.zeros((128, G, 128))
    for g in range(G):
        Upad[MS * g:MS * g + NB, g] = U
        Vpad[MS * g:MS * g + NB, g] = V
        Zpad[:, g, N_PART * g:N_PART * (g + 1)] = Z.T   # groups contiguous

    sig = np.asarray(sigma, dtype=np.float64)
    s2 = -0.5 * (sig[_PERM] ** 2)                       # per-mu, our order
    s2c = np.broadcast_to(s2, (P, NB)).copy()           # replicated to partitions

    return (Upad.astype(np.float16).reshape(128, G * RHO),
            Vpad.astype(np.float16).reshape(128, G * RHO),
            Zpad.astype(np.float16).reshape(128, G * 128),
            s2c.astype(np.float32))


# ---------------------------------------------------------------------------
# device program
# ---------------------------------------------------------------------------

_PROGRAM = None


def _build_program():
    import concourse.bacc as bacc
    import concourse.tile as tile
    import concourse.mybir as mybir
    from concourse._compat import axon_active

    dt = mybir.dt
    f32 = dt.float32
    f16 = dt.float16
    Alu = mybir.AluOpType
    ActF = mybir.ActivationFunctionType

    nc = bacc.Bacc(
        "TRN2",
        target_bir_lowering=False,
        debug=not axon_active(),
        num_devices=N_CORES,
    )

    x_d = nc.dram_tensor("x", [P, T * SDIM], f32, kind="ExternalInput")
    s2c_d = nc.dram_tensor("s2c", [P, NB], f32, kind="ExternalInput")
    id_d = nc.dram_tensor("ident", [P, P], f16, kind="ExternalInput")
    u_d = nc.dram_tensor("u", [128, G * RHO], f16, kind="ExternalInput")
    v_d = nc.dram_tensor("v", [128, G * RHO], f16, kind="ExternalInput")
    z_d = nc.dram_tensor("z", [128, G * 128], f16, kind="ExternalInput")
    out_d = nc.dram_tensor("out_t", [G * N_PART, 2 * 448], f32, kind="ExternalOutput")

    with tile.TileContext(nc) as tc:
        with (
            tc.tile_pool(name="sb", bufs=1) as sb,
            tc.tile_pool(name="ps", bufs=3, space="PSUM") as ps,
            tc.tile_pool(name="po", bufs=1, space="PSUM") as po,
        ):
            htab = sb.tile([P, 4 * T * SDIM], f32, tag="htab")   # (n:4)(t:32)(d:3)
            x2 = sb.tile([P, T * SDIM], f32, tag="x2")
            rho_t = sb.tile([P, T], f32, tag="rho")
            s2c = sb.tile([P, NB], f32, tag="s2c")
            ident = sb.tile([P, P], f16, tag="ident")
            u16 = sb.tile([128, G * RHO], f16, tag="u16")
            v16 = sb.tile([128, G * RHO], f16, tag="v16")
            z16 = sb.tile([128, G * 128], f16, tag="z16")
            hprod = sb.tile([P, T * MS], f32, tag="hprod")       # (t)(m:32)
            earg = sb.tile([P, T * NB], f32, tag="earg")         # (t)(20)
            env = sb.tile([P, T * NB], f32, tag="env")
            basis = sb.tile([P, T * MS], f16, tag="basis")       # (t)(m:32)
            stv = sb.tile([128, 2 * G * 8], f32, tag="stv")      # (h:2)(cs:4)(b:8)
            at16 = sb.tile([128, 2 * 448], f16, tag="at16")      # (h:2)(cs:4)(112)
            bt16 = sb.tile([128, 2 * 448], f16, tag="bt16")
            gb16 = sb.tile([128, 3 * 896], f16, tag="gb16")      # triple buffer
            t16 = sb.tile([128, G * 896], f16, tag="t16")        # (g:4)(h:2)(448)
            osb = sb.tile([128, 2 * 448], f32, tag="osb")        # (h:2)(448)
            wu_w = sb.tile([128, 128], f16, tag="wu_w")
            wu_r = sb.tile([128, 512], f16, tag="wu_r")

            # ---- input DMAs ---------------------------------------------
            # x via the gpsimd software DGE: the GP engine is free right
            # after the start barrier, HWDGE queues only get going ~2us
            # later (scalar is also blocked by the hoisted ACT table load).
            h4 = htab[:].rearrange("p (n t d) -> p n t d", n=4, t=T, d=SDIM)
            xv_src = x_d[:].rearrange("p (t d) -> p t d", t=T, d=SDIM)
            nc.sync.dma_start(h4[0:56, 1], xv_src[0:56])
            nc.gpsimd.dma_start(h4[56:P, 1], xv_src[56:P])
            nc.sync.dma_start(s2c[:], s2c_d[:])
            nc.sync.dma_start(v16[:], v_d[:])
            nc.sync.dma_start(u16[:], u_d[:])
            nc.sync.dma_start(z16[:], z_d[:])
            nc.sync.dma_start(ident[:], id_d[:])

            # ---- memsets (warm-up operands on DVE so the PE can start
            # while gpsimd is still issuing the x DMA) --------------------
            nc.vector.memset(wu_w[:], 1.0)
            nc.vector.memset(wu_r[:], 1.0)
            nc.gpsimd.memset(h4[:, 0], 1.0)
            hp = hprod[:].rearrange("p (t m) -> p t m", t=T, m=MS)
            bb4 = basis[:].rearrange("p (t m) -> p t m", t=T, m=MS)
            nc.gpsimd.memset(hp[:, :, 0], 1.0)
            nc.gpsimd.memset(bb4[:, :, NB:MS], 0.0)   # pad mu slots -> 0

            # ---- PE warm-up (fp16, full-tile like the real matmuls) -----
            wu_p = ps.tile([128, 1024], f32, tag="ps")
            for wi in range(NWU):
                nc.tensor.matmul(wu_p[:, 0:512], wu_w[:], wu_r[:],
                                 start=True, stop=True)

            # ---- hermite table -----------------------------------------
            x_ap = h4[:, 1]
            x2v = x2[:].rearrange("p (t d) -> p t d", t=T, d=SDIM)
            nc.vector.tensor_tensor(x2v, x_ap, x_ap, op=Alu.mult)
            nc.vector.tensor_reduce(rho_t[:], x2v, axis=mybir.AxisListType.X,
                                    op=Alu.add)
            # envelope argument early: the rho -> earg -> exp -> basis chain
            # is the critical path (exp runs on ACT, in parallel with DVE)
            ea = earg[:].rearrange("p (t m) -> p t m", t=T, m=NB)
            ev = env[:].rearrange("p (t m) -> p t m", t=T, m=NB)
            TH = T // 2
            # h2' = x^2 - 0.5   (H2 = 4x^2-2 = 4*h2')
            nc.vector.tensor_scalar_sub(h4[:, 2], x2v, 0.5)
            # h3' = (x^2 - 1.5)*x   (H3 = 8x^3-12x = 8*h3')
            nc.vector.scalar_tensor_tensor(h4[:, 3], x2v, 1.5, x_ap,
                                           op0=Alu.subtract, op1=Alu.mult)
            # mu1..3 copy queued on ACT before the exps: it gates DVE's
            # x0-product chain, the exps are only needed later by bb
            nc.scalar.copy(hp[:, :, 1:4], h4[:, 1:4, :, 2].transpose([0, 2, 1]))
            for h in range(2):
                ts = slice(TH * h, TH * (h + 1))
                eng = nc.vector if h == 0 else nc.gpsimd
                eng.tensor_tensor(
                    ea[:, ts],
                    rho_t[:, ts].unsqueeze(-1).broadcast_to((P, TH, NB)),
                    s2c[:].unsqueeze(1).broadcast_to((P, TH, NB)),
                    op=Alu.mult)
                nc.scalar.activation(ev[:, ts], ea[:, ts], ActF.Exp)

            # ---- pair products into hprod[:, :, 0:10] -------------------
            # mu4..6: h1(x1) * {1, h1(x2), h2'(x2)}
            x1h1 = h4[:, 1, :, 1].unsqueeze(-1).broadcast_to((P, T, 3))
            nc.vector.tensor_tensor(hp[:, :, 4:7], x1h1,
                                    h4[:, 0:3, :, 2].transpose([0, 2, 1]),
                                    op=Alu.mult)
            # mu7..8: h2'(x1) * {1, h1(x2)}
            x1h2 = h4[:, 2, :, 1].unsqueeze(-1).broadcast_to((P, T, 2))
            nc.vector.tensor_tensor(hp[:, :, 7:9], x1h2,
                                    h4[:, 0:2, :, 2].transpose([0, 2, 1]),
                                    op=Alu.mult)
            # mu9: h3'(x1)
            nc.gpsimd.tensor_copy(hp[:, :, 9], h4[:, 3, :, 1])

            # ---- x0 products into hprod[:, :, 10:20] --------------------
            x0h1 = h4[:, 1, :, 0].unsqueeze(-1)
            nc.vector.tensor_tensor(hp[:, :, 10:13],
                                    x0h1.broadcast_to((P, T, 3)),
                                    hp[:, :, 0:3], op=Alu.mult)
            nc.vector.tensor_tensor(hp[:, :, 13:15],
                                    x0h1.broadcast_to((P, T, 2)),
                                    hp[:, :, 4:6], op=Alu.mult)
            nc.vector.tensor_tensor(hp[:, :, 15], x0h1.squeeze(-1),
                                    hp[:, :, 7], op=Alu.mult)
            x0h2 = h4[:, 2, :, 0].unsqueeze(-1)
            nc.vector.tensor_tensor(hp[:, :, 16:18],
                                    x0h2.broadcast_to((P, T, 2)),
                                    hp[:, :, 0:2], op=Alu.mult)
            nc.vector.tensor_tensor(hp[:, :, 18], x0h2.squeeze(-1),
                                    hp[:, :, 4], op=Alu.mult)
            nc.gpsimd.tensor_copy(hp[:, :, 19], h4[:, 3, :, 0])

            # ---- basis = hprod * env (fp16, per h-half) -----------------
            for h in range(2):
                ts = slice(TH * h, TH * (h + 1))
                nc.vector.tensor_tensor(bb4[:, ts, 0:NB], hp[:, ts, 0:NB],
                                        ev[:, ts], op=Alu.mult)

            # ---- PE transpose (fp16: 1 cyc/row, one PSUM bank) ----------
            # chunk cc covers basis cols [128cc, 128cc+128) = tiles 4cc..4cc+3
            # -> btp[(jb:4)(mu:32)=128p, 112*cc ..+112]  (1792B: single bank)
            sv = stv[:].rearrange("p (h c b) -> p h c b", h=2, c=G, b=8)
            av = at16[:].rearrange("p (h c q) -> p h c q", h=2, c=G, q=P)
            bv = bt16[:].rearrange("p (h c q) -> p h c q", h=2, c=G, q=P)
            for h in range(2):
                # separate one-bank PSUM tile per half: no write-after-read
                # coupling between h1 transposes and h0's S/A consumers
                btp = ps.tile([128, 448], f16, tag="ps")
                for cs4 in range(4):
                    cc = 4 * h + cs4
                    nc.tensor.transpose(
                        btp[:, 112 * cs4:112 * cs4 + P],
                        basis[:, 128 * cc:128 * (cc + 1)],
                        ident[:])
                bsrc = btp[:].rearrange("p (c q) -> p c q", c=G, q=P)
                nc.scalar.copy(bv[:, h], bsrc)
                # the reduce reads the PSUM tile directly, so it starts right
                # after the transposes instead of after the ACT copy; the
                # subtract reads the SBUF copy, so btp's last reader is the
                # reduce and its pool slot still frees before the g1
                # B-projection needs it
                nc.vector.tensor_reduce(
                    sv[:, h],
                    bsrc.rearrange("p c (b i) -> p c b i", b=8, i=N_PART),
                    axis=mybir.AxisListType.X, op=Alu.add)
                nc.vector.tensor_tensor(
                    av[:, h].rearrange("p c (b i) -> p c b i", b=8, i=N_PART),
                    sv[:, h].unsqueeze(-1).broadcast_to((128, G, 8, N_PART)),
                    bv[:, h].rearrange("p c (b i) -> p c b i", b=8, i=N_PART),
                    op=Alu.subtract)
            # keep the PE clock hot through the S/A window
            for _ in range(NWU2):
                nc.tensor.matmul(wu_p[:, 0:512], wu_w[:], wu_r[:],
                                 start=True, stop=True)

            # ---- rank projections, product, output projection -----------
            # all weights are full 128x128 zero-banded tiles; the four output
            # groups accumulate into one PSUM tile (bank per h).
            o_ps = po.tile([128, 1024], f32, tag="po")

            def b_proj(g):
                b_ps = ps.tile([128, 1024], f32, tag="ps")
                for h in range(2):
                    cs = slice(448 * h, 448 * (h + 1))
                    nc.tensor.matmul(b_ps[:, 512 * h:512 * h + 448],
                                     v16[:, RHO * g:RHO * (g + 1)], bt16[:, cs],
                                     start=True, stop=True)
                gbv = gb16[:, 896 * (g % 3):896 * (g % 3) + 896].rearrange(
                    "p (h q) -> p h q", h=2, q=448)
                bp2 = b_ps[:].rearrange("p (h q) -> p h q", h=2, q=512)[:, :, 0:448]
                nc.scalar.copy(gbv, bp2)
                return gbv

            # B-projections run two groups ahead so the ACT drains pipeline
            # in front of the DVE products instead of starving them
            gbs = [b_proj(0), b_proj(1)]
            for g in range(G):
                a_ps = ps.tile([128, 1024], f32, tag="ps")
                for h in range(2):
                    cs = slice(448 * h, 448 * (h + 1))
                    nc.tensor.matmul(a_ps[:, 512 * h:512 * h + 448],
                                     u16[:, RHO * g:RHO * (g + 1)], at16[:, cs],
                                     start=True, stop=True)
                if g + 2 < G:
                    gbs.append(b_proj(g + 2))
                ap2 = a_ps[:].rearrange("p (h q) -> p h q", h=2, q=512)[:, :, 0:448]
                tg = t16[:, 896 * g:896 * (g + 1)].rearrange(
                    "p (h q) -> p h q", h=2, q=448)
                nc.vector.tensor_tensor(tg, ap2, gbs[g], op=Alu.mult)
                # out^T: the four groups accumulate (banded Z columns)
                for h in range(2):
                    nc.tensor.matmul(o_ps[:, 512 * h:512 * h + 448],
                                     z16[:, 128 * g:128 * (g + 1)],
                                     t16[:, 896 * g + 448 * h:][:, 0:448],
                                     start=(g == 0), stop=(g == G - 1))

            # ---- drain + store ------------------------------------------
            # group outputs live at contiguous partitions 14g..14g+13, so a
            # single 56-partition drain and two contiguous-partition DMA
            # pushes cover the whole output
            GO = G * N_PART
            ov = o_ps[0:GO].rearrange("p (h q) -> p h q", h=2, q=512)[:, :, 0:448]
            osv = osb[0:GO].rearrange("p (h q) -> p h q", h=2, q=448)
            nc.vector.tensor_copy(osv[:, 0], ov[:, 0])
            nc.scalar.copy(osv[:, 1], ov[:, 1])
            nc.sync.dma_start(out_d[0:GO // 2], osb[0:GO // 2, :])
            nc.scalar.dma_start(out_d[GO // 2:GO], osb[GO // 2:GO, :])

    nc.compile()
    return nc


def _get_program():
    global _PROGRAM
    if _PROGRAM is None:
        _PROGRAM = _build_program()
    return _PROGRAM


# ---------------------------------------------------------------------------
# entry point
# ---------------------------------------------------------------------------

LAST_RESULTS = None


def kernel(ri, rij_dist=None, sigma=None, coeff=None, **_unused):
    import os
    from concourse.bass_utils import run_bass_kernel_spmd

    global LAST_RESULTS
    ri = np.ascontiguousarray(np.asarray(ri, dtype=np.float32))
    U, V, Zd, s2c = _decompose(coeff, sigma)
    ident = np.eye(P, dtype=np.float16)

    nc = _get_program()
    in_maps = []
    for i in range(N_CORES):
        chunk = ri[i * BC:(i + 1) * BC].reshape(T, P, SDIM)
        x = np.ascontiguousarray(chunk.transpose(1, 0, 2).reshape(P, T * SDIM))
        in_maps.append({
            "x": x, "s2c": s2c, "u": U, "v": V, "z": Zd, "ident": ident,
        })

    trace = bool(int(os.environ.get("BOB_TRACE", "0")))
    res = run_bass_kernel_spmd(nc, in_maps, core_ids=list(range(N_CORES)),
                               trace=trace)
    LAST_RESULTS = res

    outs = []
    for i in range(N_CORES):
        ot = res.results[i]["out_t"]                      # (56, 896)
        # row = 14g+o; col c = (h:2)(cs:4)(p:112); tile t = 16h + 4cs + g
        arr = ot.reshape(G, N_PART, 2, 4, P).transpose(2, 3, 0, 4, 1)
        outs.append(arr.reshape(R, N_PART).reshape(BC, N_PART, N_PART))
    return np.ascontiguousarray(np.concatenate(outs, axis=0), dtype=np.float32)


# revision 27
# speedup vs baseline: 1.0736x; 1.0211x over previous
"""Trainium2 Bass kernel for nn_BasisOrbitalBackflow.

Math (reference collapses the N x N pair pooling):
    chi[b,i,mu]   = hermite_prod(ri[b,i], mu) * exp(-0.5 sigma_mu^2 |ri[b,i]|^2)
    S[b,mu]       = sum_i chi[b,i,mu]
    A[b,i,p]      = S[b,p] - chi[b,i,p]
    out[b,i,o]    = sum_{p,q} A[b,i,p] chi[b,i,q] C[p,q,o] / (N-1)

Device strategy (pure data parallel over batch, 8 cores, 256 batches each):
    * C (permuted/scaled) compressed to a rank-128 CP decomposition via ALS
      (generic rank of a 20x20x14 tensor is ~108, so rank 128 fits to ~1e-6):
      C[p,q,o] ~= sum_m U[p,m] V[q,m] Z[m,o]
      -> out^T = Z^T @ ((U^T A^T) * (V^T B^T)); one 128-wide rho chunk.
    * fp16 matmul path; rel err vs the f64 reference ~1e-3 (gate is 2e-2).
    * basis chi built in fp16 on DVE [112 partitions, 32 tiles x 32 mu-slots]
      (Hermite polys rescaled by powers of two; scale folded into C)
    * PE transposes (fp16: 1 cyc/row) move basis into one-bank fp16 PSUM
      tiles [(jb:4)(mu:32)=128p, (cs:4)(112)], one tile per half so the h1
      transposes don't serialize behind h0's consumers; ACT drains B^T to
      fp16 SBUF
    * S = segment-reduce over i (f32 accum), A^T = S_bcast - B^T -> fp16
    * all matmuls use full 128x128 zero-banded weights (no tile_position —
      small weight tiles run at half the column rate on this hardware):
      per group g, U_g/V_g have rows outside [32g, 32g+20) zeroed; Z_g has
      cols outside [32g, 32g+14) zeroed and the four groups accumulate into
      one PSUM tile.
    * ACT drains the V-side PSUM to fp16 SBUF, DVE forms t = GA * GB (fp16)
    * one packed [128, 896] drain (V+ACT halves), output DMA per group pair
"""

import itertools
import numpy as np

N_MAX = 3
SDIM = 3
N_PART = 14
BATCH = 2048
NB = 20
N_CORES = 8
BC = BATCH // N_CORES          # 256 batches per core
R = BC * N_PART                # 3584 rows per core
P = 112                        # rows per tile (8 batches)
T = R // P                     # 32 tiles
G = 4                          # transposed-layout groups (jb)
MS = 32                        # mu slot stride (20 real + 12 pad)
RHO = 128                      # CP rank (one PE chunk)
NWU = 12                       # PE clock warm-up matmuls
NWU2 = 4                       # gap fillers: PE pstate decays during idles
ALS_ITERS = 1200

# ---------------------------------------------------------------------------
# host-side constant construction
# ---------------------------------------------------------------------------

# reference mu ordering (sorted by |n|, stable)
_NS_REF = [tuple(n) for n in sorted(
    (n for n in itertools.product(range(N_MAX + 1), repeat=SDIM) if sum(n) <= N_MAX),
    key=sum)]

# our mu ordering, chosen so the product assembly uses few strided DVE ops:
# the pairs multiplied by h1(x0) sit at [0:6] and those by h2(x0) at [0:3],
# so the x0 stage is two contiguous tensor_tensor ops.
_PAIRS = [(0, 0), (1, 0), (0, 1), (2, 0), (0, 2), (1, 1), (3, 0), (0, 3), (1, 2), (2, 1)]
_NS_OURS = ([(0,) + pr for pr in _PAIRS]
            + [(1,) + _PAIRS[k] for k in range(6)]
            + [(2,) + _PAIRS[k] for k in range(3)]
            + [(3, 0, 0)])
assert sorted(_NS_OURS) == sorted(_NS_REF) and len(_NS_OURS) == NB
_PERM = np.array([_NS_REF.index(n) for n in _NS_OURS], dtype=np.int64)  # ours -> ref
_ABS_N = np.array([sum(n) for n in _NS_OURS], dtype=np.float64)


def _cp_als(C, rank, iters, seed=0, reg=1e-12):
    """Rank-`rank` CP decomposition of the (20,20,14) tensor C by ALS."""
    rng = np.random.default_rng(seed)
    U = rng.standard_normal((NB, rank))
    V = rng.standard_normal((NB, rank))
    Z = rng.standard_normal((N_PART, rank))
    C1 = C.reshape(NB, NB * N_PART)
    C2 = C.transpose(1, 0, 2).reshape(NB, NB * N_PART)
    C3 = C.transpose(2, 0, 1).reshape(N_PART, NB * NB)
    eye = reg * np.eye(rank)
    for _ in range(iters):
        KR = (V[:, None, :] * Z[None, :, :]).reshape(NB * N_PART, rank)
        U = np.linalg.solve((V.T @ V) * (Z.T @ Z) + eye, KR.T @ C1.T).T
        KR = (U[:, None, :] * Z[None, :, :]).reshape(NB * N_PART, rank)
        V = np.linalg.solve((U.T @ U) * (Z.T @ Z) + eye, KR.T @ C2.T).T
        KR = (U[:, None, :] * V[None, :, :]).reshape(NB * NB, rank)
        Z = np.linalg.solve((U.T @ U) * (V.T @ V) + eye, KR.T @ C3.T).T
        nu = np.linalg.norm(U, axis=0)
        nv = np.linalg.norm(V, axis=0)
        nz = np.linalg.norm(Z, axis=0)
        g = np.cbrt(nu * nv * nz)
        U *= g / nu
        V *= g / nv
        Z *= g / nz
    fit = np.linalg.norm(np.einsum('pr,qr,or->pqo', U, V, Z) - C) / np.linalg.norm(C)
    return U, V, Z, fit


def _decompose(coeff, sigma):
    """Build all device constants from the (400,14) coeff and (20,) sigma."""
    C = np.asarray(coeff, dtype=np.float64).reshape(NB, NB, N_PART)
    # permute to our mu order; ALS runs on the unscaled tensor (converges to
    # ~1e-6 there), then the 2^{|n|} Hermite rescale folds exactly into the
    # U/V factor rows afterwards.
    C = C[np.ix_(_PERM, _PERM)] / (N_PART - 1)

    U, V, Z, fit = _cp_als(C, RHO, ALS_ITERS)
    if fit > 5e-4:  # rare: try more iterations / another seed
        U, V, Z, fit = _cp_als(C, RHO, 4 * ALS_ITERS, seed=1)
    scale = 2.0 ** _ABS_N
    U = U * scale[:, None]
    V = V * scale[:, None]

    # per-group zero-banded full 128x128 weights, packed [128, (g:4)(128)]
    # on-device (per-partition contiguous on the DRAM side)
    Upad = np.zeros((128, G, RHO))
    Vpad = np.zeros((128, G, RHO))
    Zpad = np.zeros((128, G, 128))
    for g in range(G):
        Upad[MS * g:MS * g + NB, g] = U
        Vpad[MS * g:MS * g + NB, g] = V
        Zpad[:, g, N_PART * g:N_PART * (g + 1)] = Z.T   # groups contiguous

    sig = np.asarray(sigma, dtype=np.float64)
    s2 = -0.5 * (sig[_PERM] ** 2)                       # per-mu, our order
    s2c = np.broadcast_to(s2, (P, NB)).copy()           # replicated to partitions

    return (Upad.astype(np.float16).reshape(128, G * RHO),
            Vpad.astype(np.float16).reshape(128, G * RHO),
            Zpad.astype(np.float16).reshape(128, G * 128),
            s2c.astype(np.float32))


# ---------------------------------------------------------------------------
# device program
# ---------------------------------------------------------------------------

_PROGRAM = None


def _build_program():
    import concourse.bacc as bacc
    import concourse.tile as tile
    import concourse.mybir as mybir
    from concourse._compat import axon_active

    dt = mybir.dt
    f32 = dt.float32
    f16 = dt.float16
    Alu = mybir.AluOpType
    ActF = mybir.ActivationFunctionType

    nc = bacc.Bacc(
        "TRN2",
        target_bir_lowering=False,
        debug=not axon_active(),
        num_devices=N_CORES,
    )

    x_d = nc.dram_tensor("x", [P, T * SDIM], f32, kind="ExternalInput")
    s2c_d = nc.dram_tensor("s2c", [P, NB], f32, kind="ExternalInput")
    id_d = nc.dram_tensor("ident", [P, P], f16, kind="ExternalInput")
    u_d = nc.dram_tensor("u", [128, G * RHO], f16, kind="ExternalInput")
    v_d = nc.dram_tensor("v", [128, G * RHO], f16, kind="ExternalInput")
    z_d = nc.dram_tensor("z", [128, G * 128], f16, kind="ExternalInput")
    out_d = nc.dram_tensor("out_t", [G * N_PART, 2 * 448], f32, kind="ExternalOutput")

    with tile.TileContext(nc) as tc:
        with (
            tc.tile_pool(name="sb", bufs=1) as sb,
            tc.tile_pool(name="ps", bufs=3, space="PSUM") as ps,
            tc.tile_pool(name="po", bufs=1, space="PSUM") as po,
        ):
            htab = sb.tile([P, 4 * T * SDIM], f32, tag="htab")   # (n:4)(t:32)(d:3)
            x2 = sb.tile([P, T * SDIM], f32, tag="x2")
            rho_t = sb.tile([P, T], f32, tag="rho")
            s2c = sb.tile([P, NB], f32, tag="s2c")
            ident = sb.tile([P, P], f16, tag="ident")
            u16 = sb.tile([128, G * RHO], f16, tag="u16")
            v16 = sb.tile([128, G * RHO], f16, tag="v16")
            z16 = sb.tile([128, G * 128], f16, tag="z16")
            hprod = sb.tile([P, T * MS], f32, tag="hprod")       # (t)(m:32)
            earg = sb.tile([P, T * NB], f32, tag="earg")         # (t)(20)
            env = sb.tile([P, T * NB], f32, tag="env")
            basis = sb.tile([P, T * MS], f16, tag="basis")       # (t)(m:32)
            stv = sb.tile([128, 2 * G * 8], f32, tag="stv")      # (h:2)(cs:4)(b:8)
            at16 = sb.tile([128, 2 * 448], f16, tag="at16")      # (h:2)(cs:4)(112)
            bt16 = sb.tile([128, 2 * 448], f16, tag="bt16")
            gb16 = sb.tile([128, 3 * 896], f16, tag="gb16")      # triple buffer
            t16 = sb.tile([128, G * 896], f16, tag="t16")        # (g:4)(h:2)(448)
            osb = sb.tile([128, 2 * 448], f32, tag="osb")        # (h:2)(448)
            wu_w = sb.tile([128, 128], f16, tag="wu_w")
            wu_r = sb.tile([128, 512], f16, tag="wu_r")

            # ---- input DMAs ---------------------------------------------
            # x via the gpsimd software DGE: the GP engine is free right
            # after the start barrier, HWDGE queues only get going ~2us
            # later (scalar is also blocked by the hoisted ACT table load).
            h4 = htab[:].rearrange("p (n t d) -> p n t d", n=4, t=T, d=SDIM)
            xv_src = x_d[:].rearrange("p (t d) -> p t d", t=T, d=SDIM)
            nc.sync.dma_start(h4[0:56, 1], xv_src[0:56])
            nc.gpsimd.dma_start(h4[56:P, 1], xv_src[56:P])
            nc.sync.dma_start(s2c[:], s2c_d[:])
            nc.sync.dma_start(v16[:], v_d[:])
            nc.sync.dma_start(u16[:], u_d[:])
            nc.sync.dma_start(z16[:], z_d[:])
            nc.sync.dma_start(ident[:], id_d[:])

            # ---- memsets (warm-up operands on DVE so the PE can start
            # while gpsimd is still issuing the x DMA) --------------------
            nc.vector.memset(wu_w[:], 1.0)
            nc.vector.memset(wu_r[:], 1.0)
            nc.gpsimd.memset(h4[:, 0], 1.0)
            hp = hprod[:].rearrange("p (t m) -> p t m", t=T, m=MS)
            bb4 = basis[:].rearrange("p (t m) -> p t m", t=T, m=MS)
            nc.gpsimd.memset(hp[:, :, 0], 1.0)
            nc.gpsimd.memset(bb4[:, :, NB:MS], 0.0)   # pad mu slots -> 0

            # ---- PE warm-up (fp16, full-tile like the real matmuls) -----
            wu_p = ps.tile([128, 1024], f32, tag="ps")
            for wi in range(NWU):
                nc.tensor.matmul(wu_p[:, 0:512], wu_w[:], wu_r[:],
                                 start=True, stop=True)

            # ---- hermite table -----------------------------------------
            x_ap = h4[:, 1]
            x2v = x2[:].rearrange("p (t d) -> p t d", t=T, d=SDIM)
            nc.vector.tensor_tensor(x2v, x_ap, x_ap, op=Alu.mult)
            nc.vector.tensor_reduce(rho_t[:], x2v, axis=mybir.AxisListType.X,
                                    op=Alu.add)
            # envelope argument early: the rho -> earg -> exp -> basis chain
            # is the critical path (exp runs on ACT, in parallel with DVE)
            ea = earg[:].rearrange("p (t m) -> p t m", t=T, m=NB)
            ev = env[:].rearrange("p (t m) -> p t m", t=T, m=NB)
            TH = T // 2
            # h2' = x^2 - 0.5   (H2 = 4x^2-2 = 4*h2')
            nc.vector.tensor_scalar_sub(h4[:, 2], x2v, 0.5)
            # h3' = (x^2 - 1.5)*x   (H3 = 8x^3-12x = 8*h3')
            nc.vector.scalar_tensor_tensor(h4[:, 3], x2v, 1.5, x_ap,
                                           op0=Alu.subtract, op1=Alu.mult)
            # mu1,2 copy queued on ACT before the exps: it gates DVE's
            # x0-product chain, the exps are only needed later by bb
            nc.scalar.copy(hp[:, :, 1:3], h4[:, 1, :, 1:3])
            for h in range(2):
                ts = slice(TH * h, TH * (h + 1))
                eng = nc.vector if h == 0 else nc.gpsimd
                eng.tensor_tensor(
                    ea[:, ts],
                    rho_t[:, ts].unsqueeze(-1).broadcast_to((P, TH, NB)),
                    s2c[:].unsqueeze(1).broadcast_to((P, TH, NB)),
                    op=Alu.mult)
                nc.scalar.activation(ev[:, ts], ea[:, ts], ActF.Exp)

            # ---- pair products into hprod[:, :, 0:10] -------------------
            # mu1,2 = h1(x1), h1(x2);  mu3,4 = h2'(x1), h2'(x2);
            # mu6,7 = h3'(x1), h3'(x2): three two-column copies (ACT/GP)
            nc.gpsimd.tensor_copy(hp[:, :, 3:5], h4[:, 2, :, 1:3])
            nc.gpsimd.tensor_copy(hp[:, :, 6:8], h4[:, 3, :, 1:3])
            # mu5 = h1(x1)h1(x2), mu8 = h1(x1)h2'(x2), mu9 = h2'(x1)h1(x2)
            nc.vector.tensor_tensor(hp[:, :, 5], h4[:, 1, :, 1],
                                    h4[:, 1, :, 2], op=Alu.mult)
            nc.vector.tensor_tensor(hp[:, :, 8], h4[:, 1, :, 1],
                                    h4[:, 2, :, 2], op=Alu.mult)
            nc.vector.tensor_tensor(hp[:, :, 9], h4[:, 2, :, 1],
                                    h4[:, 1, :, 2], op=Alu.mult)

            # ---- x0 products into hprod[:, :, 10:20] --------------------
            x0h1 = h4[:, 1, :, 0].unsqueeze(-1)
            nc.vector.tensor_tensor(hp[:, :, 10:16],
                                    x0h1.broadcast_to((P, T, 6)),
                                    hp[:, :, 0:6], op=Alu.mult)
            x0h2 = h4[:, 2, :, 0].unsqueeze(-1)
            nc.vector.tensor_tensor(hp[:, :, 16:19],
                                    x0h2.broadcast_to((P, T, 3)),
                                    hp[:, :, 0:3], op=Alu.mult)
            nc.gpsimd.tensor_copy(hp[:, :, 19], h4[:, 3, :, 0])

            # ---- basis = hprod * env (fp16, per h-half) -----------------
            for h in range(2):
                ts = slice(TH * h, TH * (h + 1))
                nc.vector.tensor_tensor(bb4[:, ts, 0:NB], hp[:, ts, 0:NB],
                                        ev[:, ts], op=Alu.mult)

            # ---- PE transpose (fp16: 1 cyc/row, one PSUM bank) ----------
            # chunk cc covers basis cols [128cc, 128cc+128) = tiles 4cc..4cc+3
            # -> btp[(jb:4)(mu:32)=128p, 112*cc ..+112]  (1792B: single bank)
            sv = stv[:].rearrange("p (h c b) -> p h c b", h=2, c=G, b=8)
            av = at16[:].rearrange("p (h c q) -> p h c q", h=2, c=G, q=P)
            bv = bt16[:].rearrange("p (h c q) -> p h c q", h=2, c=G, q=P)
            for h in range(2):
                # separate one-bank PSUM tile per half: no write-after-read
                # coupling between h1 transposes and h0's S/A consumers
                btp = ps.tile([128, 448], f16, tag="ps")
                for cs4 in range(4):
                    cc = 4 * h + cs4
                    nc.tensor.transpose(
                        btp[:, 112 * cs4:112 * cs4 + P],
                        basis[:, 128 * cc:128 * (cc + 1)],
                        ident[:])
                bsrc = btp[:].rearrange("p (c q) -> p c q", c=G, q=P)
                nc.scalar.copy(bv[:, h], bsrc)
                # the reduce reads the PSUM tile directly, so it starts right
                # after the transposes instead of after the ACT copy; the
                # subtract reads the SBUF copy, so btp's last reader is the
                # reduce and its pool slot still frees before the g1
                # B-projection needs it
                nc.vector.tensor_reduce(
                    sv[:, h],
                    bsrc.rearrange("p c (b i) -> p c b i", b=8, i=N_PART),
                    axis=mybir.AxisListType.X, op=Alu.add)
                nc.vector.tensor_tensor(
                    av[:, h].rearrange("p c (b i) -> p c b i", b=8, i=N_PART),
                    sv[:, h].unsqueeze(-1).broadcast_to((128, G, 8, N_PART)),
                    bv[:, h].rearrange("p c (b i) -> p c b i", b=8, i=N_PART),
                    op=Alu.subtract)
            # keep the PE clock hot through the S/A window
            for _ in range(NWU2):
                nc.tensor.matmul(wu_p[:, 0:512], wu_w[:], wu_r[:],
                                 start=True, stop=True)

            # ---- rank projections, product, output projection -----------
            # all weights are full 128x128 zero-banded tiles; the four output
            # groups accumulate into one PSUM tile (bank per h).
            o_ps = po.tile([128, 1024], f32, tag="po")

            def b_proj(g):
                b_ps = ps.tile([128, 1024], f32, tag="ps")
                for h in range(2):
                    cs = slice(448 * h, 448 * (h + 1))
                    nc.tensor.matmul(b_ps[:, 512 * h:512 * h + 448],
                                     v16[:, RHO * g:RHO * (g + 1)], bt16[:, cs],
                                     start=True, stop=True)
                gbv = gb16[:, 896 * (g % 3):896 * (g % 3) + 896].rearrange(
                    "p (h q) -> p h q", h=2, q=448)
                bp2 = b_ps[:].rearrange("p (h q) -> p h q", h=2, q=512)[:, :, 0:448]
                nc.scalar.copy(gbv, bp2)
                return gbv

            # B-projections run two groups ahead so the ACT drains pipeline
            # in front of the DVE products instead of starving them
            gbs = [b_proj(0), b_proj(1)]
            for g in range(G):
                a_ps = ps.tile([128, 1024], f32, tag="ps")
                for h in range(2):
                    cs = slice(448 * h, 448 * (h + 1))
                    nc.tensor.matmul(a_ps[:, 512 * h:512 * h + 448],
                                     u16[:, RHO * g:RHO * (g + 1)], at16[:, cs],
                                     start=True, stop=True)
                if g + 2 < G:
                    gbs.append(b_proj(g + 2))
                ap2 = a_ps[:].rearrange("p (h q) -> p h q", h=2, q=512)[:, :, 0:448]
                tg = t16[:, 896 * g:896 * (g + 1)].rearrange(
                    "p (h q) -> p h q", h=2, q=448)
                nc.vector.tensor_tensor(tg, ap2, gbs[g], op=Alu.mult)
                # out^T: the four groups accumulate (banded Z columns)
                for h in range(2):
                    nc.tensor.matmul(o_ps[:, 512 * h:512 * h + 448],
                                     z16[:, 128 * g:128 * (g + 1)],
                                     t16[:, 896 * g + 448 * h:][:, 0:448],
                                     start=(g == 0), stop=(g == G - 1))

            # ---- drain + store ------------------------------------------
            # group outputs live at contiguous partitions 14g..14g+13, so a
            # single 56-partition drain and two contiguous-partition DMA
            # pushes cover the whole output
            GO = G * N_PART
            ov = o_ps[0:GO].rearrange("p (h q) -> p h q", h=2, q=512)[:, :, 0:448]
            osv = osb[0:GO].rearrange("p (h q) -> p h q", h=2, q=448)
            nc.vector.tensor_copy(osv[:, 0], ov[:, 0])
            nc.scalar.copy(osv[:, 1], ov[:, 1])
            nc.sync.dma_start(out_d[0:GO // 2], osb[0:GO // 2, :])
            nc.scalar.dma_start(out_d[GO // 2:GO], osb[GO // 2:GO, :])

    nc.compile()
    return nc


def _get_program():
    global _PROGRAM
    if _PROGRAM is None:
        _PROGRAM = _build_program()
    return _PROGRAM


# ---------------------------------------------------------------------------
# entry point
# ---------------------------------------------------------------------------

LAST_RESULTS = None


def kernel(ri, rij_dist=None, sigma=None, coeff=None, **_unused):
    import os
    from concourse.bass_utils import run_bass_kernel_spmd

    global LAST_RESULTS
    ri = np.ascontiguousarray(np.asarray(ri, dtype=np.float32))
    U, V, Zd, s2c = _decompose(coeff, sigma)
    ident = np.eye(P, dtype=np.float16)

    nc = _get_program()
    in_maps = []
    for i in range(N_CORES):
        chunk = ri[i * BC:(i + 1) * BC].reshape(T, P, SDIM)
        x = np.ascontiguousarray(chunk.transpose(1, 0, 2).reshape(P, T * SDIM))
        in_maps.append({
            "x": x, "s2c": s2c, "u": U, "v": V, "z": Zd, "ident": ident,
        })

    trace = bool(int(os.environ.get("BOB_TRACE", "0")))
    res = run_bass_kernel_spmd(nc, in_maps, core_ids=list(range(N_CORES)),
                               trace=trace)
    LAST_RESULTS = res

    outs = []
    for i in range(N_CORES):
        ot = res.results[i]["out_t"]                      # (56, 896)
        # row = 14g+o; col c = (h:2)(cs:4)(p:112); tile t = 16h + 4cs + g
        arr = ot.reshape(G, N_PART, 2, 4, P).transpose(2, 3, 0, 4, 1)
        outs.append(arr.reshape(R, N_PART).reshape(BC, N_PART, N_PART))
    return np.ascontiguousarray(np.concatenate(outs, axis=0), dtype=np.float32)
